# revision 1
# baseline (speedup 1.0000x reference)
"""Trainium2 Bass kernel for nn_Complex_Loss (complex regression loss).

Reference semantics (see problem):
    D = 4096; out/g_t: [B=16384, 2D=8192] f32, first half real, second imag.
    err = g_t - out ; sq = err_r^2 + err_i^2            [B, D]
    e_max = max_j sqrt(sq)  (== sqrt(max_j sq))         [B]
    correct = argmax(gt_r) == argmax(out_r)             [B]
    masked = correct & (e_max < e_thresh)
    sum_sq = sum over rows not masked of sum_j sq
    count = 1 + sum(masked ? 1 : D)
    return sum_sq / count

Strategy: data-parallel over the batch axis across 8 NeuronCores
(2048 rows per core).  On each core, per 128-row tile:
  - DMA gt/ot tiles in
  - GpSimd: err = gt - ot (per col-chunk, sign of err is irrelevant
    downstream since err only feeds Square)
  - ScalarE: sq = Square(err) -> bf16, accum_out = f32 row sums
  - VectorE: sq_r + sq_i (bf16 2x), row max; max8+max_index argmax on
    the f32 real halves
Per-row stats (partial row sums, row max of sq, argmax indices) are
DMA'd out; the final tiny reduction over 16384 rows (mask + scalar
sum / count / divide) happens on the host in numpy.
"""
import numpy as np
from contextlib import ExitStack

import concourse.bass as bass
import concourse.tile as tile
from concourse import bacc, mybir
from concourse.bass_utils import run_bass_kernel_spmd

# Problem shape (hardcoded per the task contract).
B, TWO_D = 16384, 8192
D = TWO_D // 2            # 4096
N_CORES = 8
R = B // N_CORES          # 2048 rows per core
P = 128                   # SBUF partitions
NT = R // P               # 16 row-tiles per core
WC = 2048                 # column chunk width (per half)
NC_CHUNK = D // WC        # 2 chunks per half

f32 = mybir.dt.float32
bf16 = mybir.dt.bfloat16
u32 = mybir.dt.uint32
Alu = mybir.AluOpType
Ax = mybir.AxisListType
Act = mybir.ActivationFunctionType

# stats_f columns: [sum_r_c0, sum_r_c1, sum_i_c0, sum_i_c1, maxsq_c0, maxsq_c1, 0, 0]
NSF = 8

_NC_CACHE = {}


def _build_nc(n_loop=0):
    """Build the per-core program.  n_loop>0 wraps the whole body in a
    hardware For_i loop (used only for timing measurements)."""
    nc = bacc.Bacc("TRN2", target_bir_lowering=False, debug=False,
                   num_devices=N_CORES)
    g = nc.dram_tensor("g", [R, TWO_D], f32, kind="ExternalInput").ap()
    o = nc.dram_tensor("o", [R, TWO_D], f32, kind="ExternalInput").ap()
    stats_f = nc.dram_tensor("stats_f", [R, NSF], f32, kind="ExternalOutput").ap()
    stats_i = nc.dram_tensor("stats_i", [R, 2], u32, kind="ExternalOutput").ap()

    with tile.TileContext(nc) as tc, ExitStack() as ctx:
        iop = ctx.enter_context(tc.tile_pool(name="io", bufs=2))
        ep = ctx.enter_context(tc.tile_pool(name="err", bufs=2))
        sqp = ctx.enter_context(tc.tile_pool(name="sq", bufs=2))
        sp = ctx.enter_context(tc.tile_pool(name="st", bufs=3))

        def body():
            # One-tile software-pipeline skew: the DVE ops that depend on
            # the GpSimd->ACT sq chain of tile t are emitted during tile
            # t+1, after tile t+1's argmax ops.  Engines run their streams
            # in order, so without the skew DVE would stall behind the
            # freshest ACT output; with it, every DVE op is ready when DVE
            # reaches it.
            pending = []  # (r0, stf, sti, sq_pairs)

            def flush_pending():
                r0, stf, sti, sq_pairs = pending.pop(0)
                sqs_chunks = []
                for c, (sq_r, sq_i) in enumerate(sq_pairs):
                    sqs = sqp.tile([P, WC], bf16, tag=f"sqs{c}")
                    nc.vector.tensor_add(sqs[:], sq_r[:], sq_i[:])
                    sqs_chunks.append(sqs)
                # fold the chunk maxima: elementwise max (bf16 2x) then one
                # 1x reduce over WC instead of NC_CHUNK reduces over WC
                fold = sqp.tile([P, WC], bf16, tag="fold")
                nc.vector.tensor_max(fold[:], sqs_chunks[0][:], sqs_chunks[1][:])
                nc.vector.tensor_reduce(stf[:, 4:5], fold[:],
                                        axis=Ax.X, op=Alu.max)
                nc.vector.memset(stf[:, 5:NSF], 0.0)
                nc.sync.dma_start(stats_f[r0:r0 + P, :], stf[:])
                nc.sync.dma_start(stats_i[r0:r0 + P, :], sti[:])

            for t in range(NT):
                r0 = t * P
                gt_ = iop.tile([P, TWO_D], f32, tag="g")
                ot_ = iop.tile([P, TWO_D], f32, tag="o")
                # Split loads: real halves first so the argmax ops (which
                # only need cols [0:D)) can start before the imag halves
                # land.  Dep tracking is address-range based, so readers
                # of a slice wait only on the DMAs covering that slice.
                for c in range(NC_CHUNK):
                    c0 = c * WC
                    nc.sync.dma_start(gt_[:, c0:c0 + WC], g[r0:r0 + P, c0:c0 + WC])
                    nc.sync.dma_start(ot_[:, c0:c0 + WC], o[r0:r0 + P, c0:c0 + WC])
                for c in range(NC_CHUNK):
                    c0 = D + c * WC
                    nc.sync.dma_start(gt_[:, c0:c0 + WC], g[r0:r0 + P, c0:c0 + WC])
                    nc.sync.dma_start(ot_[:, c0:c0 + WC], o[r0:r0 + P, c0:c0 + WC])

                stf = sp.tile([P, NSF], f32, tag="stf")
                sti = sp.tile([P, 2], u32, tag="sti")

                # --- argmax path on the f32 real halves (DVE; only needs
                # the real-half DMAs, so it fills DVE while GpSimd/ACT work) ---
                m8g = sp.tile([P, 8], f32, tag="m8g")
                nc.vector.max(m8g[:], gt_[:, 0:D])
                idxg = sp.tile([P, 8], u32, tag="idxg")
                nc.vector.max_index(idxg[:], m8g[:], gt_[:, 0:D])
                m8o = sp.tile([P, 8], f32, tag="m8o")
                nc.vector.max(m8o[:], ot_[:, 0:D])
                idxo = sp.tile([P, 8], u32, tag="idxo")
                nc.vector.max_index(idxo[:], m8o[:], ot_[:, 0:D])
                nc.vector.tensor_copy(sti[:, 0:1], idxg[:, 0:1])
                nc.vector.tensor_copy(sti[:, 1:2], idxo[:, 0:1])

                # --- sq path: GpSimd subs + ACT squares per column chunk ---
                sq_pairs = []
                for c in range(NC_CHUNK):
                    c0 = c * WC
                    err_r = ep.tile([P, WC], f32, tag="err_r")
                    nc.gpsimd.tensor_sub(err_r[:], gt_[:, c0:c0 + WC],
                                         ot_[:, c0:c0 + WC])
                    err_i = ep.tile([P, WC], f32, tag="err_i")
                    nc.gpsimd.tensor_sub(err_i[:], gt_[:, D + c0:D + c0 + WC],
                                         ot_[:, D + c0:D + c0 + WC])

                    sq_r = sqp.tile([P, WC], bf16, tag="sq_r")
                    nc.scalar.activation(sq_r[:], err_r[:], Act.Square,
                                         accum_out=stf[:, c:c + 1])
                    sq_i = sqp.tile([P, WC], bf16, tag="sq_i")
                    nc.scalar.activation(sq_i[:], err_i[:], Act.Square,
                                         accum_out=stf[:, 2 + c:3 + c])
                    sq_pairs.append((sq_r, sq_i))

                pending.append((r0, stf, sti, sq_pairs))
                if len(pending) > 1:
                    flush_pending()
            while pending:
                flush_pending()

        if n_loop > 0:
            with tc.For_i(0, n_loop, 1) as _i:
                body()
        else:
            body()

    nc.compile()
    return nc


def get_nc():
    if "nc" not in _NC_CACHE:
        _NC_CACHE["nc"] = _build_nc()
    return _NC_CACHE["nc"]


def combine_stats(stats_f, stats_i, epoch):
    """Host-side tail: [B, NSF] f32 + [B, 2] u32 per-row stats -> scalar loss."""
    row_sum = stats_f[:, 0:4].astype(np.float64).sum(axis=1)
    maxsq = stats_f[:, 4]
    correct = stats_i[:, 0] == stats_i[:, 1]

    E_T_INIT = 0.5
    if int(epoch) % 10 == 0:
        e_thresh = np.float32(E_T_INIT * np.exp(-0.2))
    else:
        e_thresh = np.float32(E_T_INIT)
    e_max = np.sqrt(maxsq.astype(np.float32))
    masked = correct & (e_max < e_thresh)

    sum_sq = row_sum[~masked].sum()
    count = 1.0 + float(masked.sum()) + float((~masked).sum()) * float(D)
    return np.float32(sum_sq / count)


def kernel(out, g_t, epoch):
    out = np.asarray(out, dtype=np.float32)
    g_t = np.asarray(g_t, dtype=np.float32)
    assert out.shape == (B, TWO_D) and g_t.shape == (B, TWO_D)

    nc = get_nc()
    in_maps = [{"g": g_t[c * R:(c + 1) * R], "o": out[c * R:(c + 1) * R]}
               for c in range(N_CORES)]
    res = run_bass_kernel_spmd(nc, in_maps, list(range(N_CORES))).results
    stats_f = np.concatenate([res[c]["stats_f"] for c in range(N_CORES)], axis=0)
    stats_i = np.concatenate([res[c]["stats_i"] for c in range(N_CORES)], axis=0)
    return combine_stats(stats_f, stats_i, epoch)



# revision 4
# speedup vs baseline: 1.1874x; 1.1874x over previous
"""Trainium2 Bass kernel for nn_Complex_Loss (complex regression loss).

Reference semantics (see problem):
    D = 4096; out/g_t: [B=16384, 2D=8192] f32, first half real, second imag.
    err = g_t - out ; sq = err_r^2 + err_i^2            [B, D]
    e_max = max_j sqrt(sq)                              [B]
    correct = argmax(gt_r) == argmax(out_r)             [B]
    masked = correct & (e_max < e_thresh)
    sum_sq = sum over rows not masked of sum_j sq
    count = 1 + sum(masked ? 1 : D)
    return sum_sq / count

Strategy: data-parallel over the batch axis across 8 NeuronCores
(2048 rows per core).  The kernel is memory-bound: it must stream
2 x 2048 x 8192 x 4B = 134 MB per core, so everything is built around
keeping the DMA queues saturated:

  - per 128-row tile, TWO 4 MB DMAs (one per tensor), issued on the two
    independent HWDGE queues (g on qSPDynamicHW via nc.sync, o on
    qActDynamicHW via nc.scalar) so their per-DMA completion overheads
    overlap at SDMA packet granularity;
  - err = g - o in fp16 (DVE takes the real half, GpSimd the imag half);
  - ACT Square(err) with accum_out produces the per-row sum of squares;
  - per-row stats accumulate in one SBUF tile, ONE small DMA at the end.

Masking: e_max^2 = max_j sq_j >= rowsum/D, so any row with
rowsum >= D*e_thresh^2 is provably unmasked (for randn-scale data that
is every row by a ~38-sigma margin).  The astronomically-rare remainder
is recomputed exactly on the host from the full inputs, so kernel() is
correct for arbitrary inputs, not just the graded distribution.
"""
import numpy as np
from contextlib import ExitStack

import concourse.bass as bass
import concourse.tile as tile
from concourse import bacc, mybir
from concourse.bass_utils import run_bass_kernel_spmd

# Problem shape (hardcoded per the task contract).
B, TWO_D = 16384, 8192
D = TWO_D // 2            # 4096
N_CORES = 8
R = B // N_CORES          # 2048 rows per core
P = 128                   # SBUF partitions
NT = R // P               # 16 row-tiles per core

f32 = mybir.dt.float32
f16 = mybir.dt.float16
Alu = mybir.AluOpType
Act = mybir.ActivationFunctionType

# stats columns: [0:NT]    sum_r   (row sum of err_r^2, per tile)
#                [NT:2NT]  sum_i
NSF = 2 * NT              # 32

_NC_CACHE = {}


def _build_nc(n_loop=0, o_engine="scalar"):
    """Build the per-core program.  n_loop>0 wraps the whole body in a
    hardware For_i loop (used only for timing measurements)."""
    nc = bacc.Bacc("TRN2", target_bir_lowering=False, debug=False,
                   num_devices=N_CORES)
    g = nc.dram_tensor("g", [R, TWO_D], f32, kind="ExternalInput").ap()
    o = nc.dram_tensor("o", [R, TWO_D], f32, kind="ExternalInput").ap()
    stats = nc.dram_tensor("stats", [P, NSF], f32, kind="ExternalOutput").ap()

    o_dma = {"scalar": nc.scalar, "sync": nc.sync}[o_engine]

    with tile.TileContext(nc) as tc, ExitStack() as ctx:
        iop = ctx.enter_context(tc.tile_pool(name="io", bufs=2))
        ep = ctx.enter_context(tc.tile_pool(name="err", bufs=2))
        dp = ctx.enter_context(tc.tile_pool(name="dummy", bufs=1))
        sp = ctx.enter_context(tc.tile_pool(name="st", bufs=1))

        def body():
            stf = sp.tile([P, NSF], f32, tag="stf")
            # dummy Square output, written every tile and never read
            dum = dp.tile([P, D], f16, tag="dum")

            # Software-pipeline skew: the ACT Squares of tile t are
            # emitted after the o-DMA trigger of tile t+1 so ACT's
            # in-order stream never blocks the next load behind a
            # compute-dependency wait.
            pending = []  # (t, err_r, err_i)

            def flush_pending():
                t, err_r, err_i = pending.pop(0)
                nc.scalar.activation(dum[:], err_r[:], Act.Square,
                                     accum_out=stf[:, t:t + 1])
                nc.scalar.activation(dum[:], err_i[:], Act.Square,
                                     accum_out=stf[:, NT + t:NT + t + 1])

            for t in range(NT):
                r0 = t * P
                gt_ = iop.tile([P, TWO_D], f32, tag="g")
                ot_ = iop.tile([P, TWO_D], f32, tag="o")
                nc.sync.dma_start(gt_[:], g[r0:r0 + P, :])
                o_dma.dma_start(ot_[:], o[r0:r0 + P, :])

                if pending:
                    flush_pending()

                # err = g - o in fp16 (feeds ACT's Square+rowsum)
                err_r = ep.tile([P, D], f16, tag="err_r")
                nc.vector.tensor_sub(err_r[:], gt_[:, 0:D], ot_[:, 0:D])
                err_i = ep.tile([P, D], f16, tag="err_i")
                nc.gpsimd.tensor_sub(err_i[:], gt_[:, D:TWO_D],
                                     ot_[:, D:TWO_D])
                pending.append((t, err_r, err_i))

            while pending:
                flush_pending()
            nc.sync.dma_start(stats[:, :], stf[:])

        if n_loop > 0:
            with tc.For_i(0, n_loop, 1) as _i:
                body()
        else:
            body()

    nc.compile()
    return nc


def get_nc():
    if "nc" not in _NC_CACHE:
        _NC_CACHE["nc"] = _build_nc()
    return _NC_CACHE["nc"]


def _e_thresh(epoch):
    E_T_INIT = 0.5
    if int(epoch) % 10 == 0:
        return np.float32(E_T_INIT * np.exp(-0.2))
    return np.float32(E_T_INIT)


def combine_stats(stats, epoch, out=None, g_t=None):
    """Host-side tail: per-core [P, NSF] stats (concatenated on axis 0 to
    [N_CORES*P, NSF]) -> scalar loss.

    Row (core c, tile t, partition p) = c*R + t*P + p maps to
    stats[c*P + p, col + t] for col in {0, NT, 2NT, 3NT}.
    """
    stats = stats.reshape(N_CORES, P, NSF)
    sum_r = stats[:, :, 0:NT]           # [C, P, NT]
    sum_i = stats[:, :, NT:2 * NT]

    # -> [C, NT, P] -> flat row order (c, t, p)
    rowsum = (sum_r + sum_i).astype(np.float64).transpose(0, 2, 1).reshape(-1)

    thresh = _e_thresh(epoch)
    # rowsum underestimates the true sum only up to fp16 rounding of err
    # (~2^-11 relative); inflate the suspect margin accordingly.  A row is
    # suspect only if sqrt(rowsum/D) fails to clear the threshold.
    margin = 1.01
    suspect = rowsum < D * (thresh * margin) ** 2

    total = rowsum.sum()
    count = 1.0 + rowsum.size * float(D)
    if suspect.any():
        # Exact recomputation for rows the device bound cannot clear.
        assert out is not None and g_t is not None, (
            "suspect rows require the full inputs for exact recomputation")
        idx = np.nonzero(suspect)[0]
        gt_s = np.asarray(g_t[idx], dtype=np.float32)
        ot_s = np.asarray(out[idx], dtype=np.float32)
        err = gt_s - ot_s
        sq = err[:, :D] ** 2 + err[:, D:] ** 2
        e_max = np.sqrt(sq.max(axis=1))
        correct = gt_s[:, :D].argmax(axis=1) == ot_s[:, :D].argmax(axis=1)
        masked = correct & (e_max < thresh)
        exact_rowsum = sq.astype(np.float64).sum(axis=1)
        # replace the device rowsum with the exact one for suspect rows;
        # masked rows contribute 0 to sum_sq and swap D -> 1 in count.
        total += (np.where(masked, 0.0, exact_rowsum) - rowsum[idx]).sum()
        count += float(masked.sum()) * (1.0 - float(D))
    return np.float32(total / count)


def kernel(out, g_t, epoch):
    out = np.asarray(out, dtype=np.float32)
    g_t = np.asarray(g_t, dtype=np.float32)
    assert out.shape == (B, TWO_D) and g_t.shape == (B, TWO_D)

    nc = get_nc()
    in_maps = [{"g": g_t[c * R:(c + 1) * R], "o": out[c * R:(c + 1) * R]}
               for c in range(N_CORES)]
    res = run_bass_kernel_spmd(nc, in_maps, list(range(N_CORES))).results
    stats = np.concatenate([res[c]["stats"] for c in range(N_CORES)], axis=0)
    return combine_stats(stats, epoch, out=out, g_t=g_t)
